# revision 2
# baseline (speedup 1.0000x reference)
"""Data-parallel forward for nn_CerebroOriginal on 8 NeuronCores.

Sharding: pure data parallel per the problem's sharding hint — batch B=256 is
split 32/core across 8 cores, parameters are replicated. The reference's
per-tensor fake-quant (qdq) scales are GLOBAL maxima over the full batch, so a
cross-core max all-reduce (lax.pmax) runs at each quantization site; this
reproduces the reference scales exactly (max is order-independent and exact).
Everything else is per-sample and communication-free.

Falls back to single-process CPU execution if the device path fails, so the
kernel always returns a correct full-shape output.
"""
import numpy as np
import jax
import jax.numpy as jnp
from functools import partial

# CerebroOriginal dims (hardcoded per contract)
B, C, T, E, H, NB, NC = 256, 14, 8, 180, 5, 6, 4
DH = E // H
S = C * T
MLP = 4 * E
NCORES = 8
BL = B // NCORES  # 32 samples per core


def _qdq(x, axis_name):
    m = jnp.max(jnp.abs(x))
    if axis_name is not None:
        m = jax.lax.pmax(m, axis_name)
    s = jnp.maximum(m / 127.0, 1e-8)
    y = jnp.clip(jnp.round(x / s), -128.0, 127.0) * s
    return y


def _ln(x, g, b):
    m = jnp.mean(x, axis=-1, keepdims=True)
    v = jnp.var(x, axis=-1, keepdims=True)
    return (x - m) * jax.lax.rsqrt(v + 1e-5) * g + b


def _forward(x, p, axis_name=None):
    """Forward for a local batch shard. x: [b, C, T]."""
    b = x.shape[0]
    qdq = partial(_qdq, axis_name=axis_name)

    def qlinear(t, w, bias):
        return qdq(t) @ qdq(w).T + bias

    h = jnp.transpose(qdq(x), (0, 2, 1))
    h = qlinear(h, p["proj_w"], p["proj_b"])
    h = jnp.broadcast_to(h[:, None], (b, C, T, E)).reshape(b, S, E)

    for i in range(NB):
        r1 = h
        a = qdq(_ln(h, p["ln1_g"][i], p["ln1_b"][i])).reshape(b, C, T, E)
        if i % 2 == 0:
            a = a.reshape(b * C, T, E)
            L = T
        else:
            a = jnp.transpose(a, (0, 2, 1, 3)).reshape(b * T, C, E)
            L = C
        q = qlinear(a, p["wq"][i], p["bq"][i]).reshape(-1, L, H, DH)
        k = qlinear(a, p["wk"][i], p["bk"][i]).reshape(-1, L, H, DH)
        v = qlinear(a, p["wv"][i], p["bv"][i]).reshape(-1, L, H, DH)
        scores = jnp.einsum("nlhd,nmhd->nhlm", q, k) / jnp.sqrt(jnp.float32(DH))
        att = jax.nn.softmax(scores, axis=-1)
        o = jnp.einsum("nhlm,nmhd->nlhd", att, v).reshape(-1, L, E)
        o = qlinear(o, p["wo"][i], p["bo"][i])
        if i % 2 == 0:
            o = o.reshape(b, C, T, E)
        else:
            o = jnp.transpose(o.reshape(b, T, C, E), (0, 2, 1, 3))
        h = r1 + o.reshape(b, S, E)

        r2 = h
        m = qdq(_ln(h, p["ln2_g"][i], p["ln2_b"][i]))
        m = qlinear(m, p["fc1_w"][i], p["fc1_b"][i])
        m = qdq(jax.nn.gelu(m))
        m = qlinear(m, p["fc2_w"][i], p["fc2_b"][i])
        h = r2 + m

    h = qdq(_ln(h, p["fn_g"], p["fn_b"]))
    pooled = jnp.mean(h, axis=1)
    return qlinear(pooled, p["cls_w"], p["cls_b"])


_DEVICE_FN = None


def _get_device_fn():
    global _DEVICE_FN
    if _DEVICE_FN is not None:
        return _DEVICE_FN
    from jax.sharding import Mesh, PartitionSpec as P
    from jax.experimental.shard_map import shard_map

    devs = jax.devices()[:NCORES]
    mesh = Mesh(np.asarray(devs), ("i",))

    def shard_fwd(x, p):
        return _forward(x, p, axis_name="i")

    fn = jax.jit(
        shard_map(
            shard_fwd,
            mesh=mesh,
            in_specs=(P("i"), P()),
            out_specs=P("i"),
            check_rep=False,
        )
    )
    _DEVICE_FN = fn
    return fn


def _qdq_np(x):
    s = max(np.abs(x).max() / 127.0, 1e-8)
    return np.clip(np.round(x / s), -128.0, 127.0).astype(np.float32) * np.float32(s)


def _ln_np(x, g, b):
    m = x.mean(-1, keepdims=True)
    v = x.var(-1, keepdims=True)
    return (x - m) / np.sqrt(v + 1e-5) * g + b


def _gelu_np(x):
    # jax.nn.gelu default (approximate=True): tanh approximation
    return (0.5 * x * (1.0 + np.tanh(0.7978845608028654 * (x + 0.044715 * x**3)))).astype(np.float32)


def _forward_np(x, p):
    def qlinear(t, w, bias):
        return _qdq_np(t) @ _qdq_np(w).T + bias

    h = np.transpose(_qdq_np(x), (0, 2, 1))
    h = qlinear(h, p["proj_w"], p["proj_b"])
    h = np.broadcast_to(h[:, None], (B, C, T, E)).reshape(B, S, E).astype(np.float32)
    for i in range(NB):
        r1 = h
        a = _qdq_np(_ln_np(h, p["ln1_g"][i], p["ln1_b"][i])).reshape(B, C, T, E)
        if i % 2 == 0:
            a = a.reshape(B * C, T, E)
            L = T
        else:
            a = np.transpose(a, (0, 2, 1, 3)).reshape(B * T, C, E)
            L = C
        q = qlinear(a, p["wq"][i], p["bq"][i]).reshape(-1, L, H, DH)
        k = qlinear(a, p["wk"][i], p["bk"][i]).reshape(-1, L, H, DH)
        v = qlinear(a, p["wv"][i], p["bv"][i]).reshape(-1, L, H, DH)
        scores = np.einsum("nlhd,nmhd->nhlm", q, k) / np.sqrt(np.float32(DH))
        scores -= scores.max(-1, keepdims=True)
        e = np.exp(scores)
        att = e / e.sum(-1, keepdims=True)
        o = np.einsum("nhlm,nmhd->nlhd", att.astype(np.float32), v).reshape(-1, L, E)
        o = qlinear(o, p["wo"][i], p["bo"][i])
        if i % 2 == 0:
            o = o.reshape(B, C, T, E)
        else:
            o = np.transpose(o.reshape(B, T, C, E), (0, 2, 1, 3))
        h = r1 + o.reshape(B, S, E)
        r2 = h
        m = _qdq_np(_ln_np(h, p["ln2_g"][i], p["ln2_b"][i]))
        m = qlinear(m, p["fc1_w"][i], p["fc1_b"][i])
        m = _qdq_np(_gelu_np(m))
        m = qlinear(m, p["fc2_w"][i], p["fc2_b"][i])
        h = (r2 + m).astype(np.float32)
    h = _qdq_np(_ln_np(h, p["fn_g"], p["fn_b"]))
    pooled = h.mean(1)
    return qlinear(pooled, p["cls_w"], p["cls_b"])


def kernel(**inputs):
    x = np.asarray(inputs["x"], dtype=np.float32)
    p = {k: np.asarray(v) for k, v in inputs.items() if k != "x"}
    try:
        devs = jax.devices()
        if len(devs) >= NCORES and devs[0].platform != "cpu":
            fn = _get_device_fn()
            out = fn(jnp.asarray(x), jax.tree.map(jnp.asarray, p))
            out = np.asarray(jax.block_until_ready(out))
            if out.shape == (B, NC) and np.all(np.isfinite(out)):
                return out.astype(np.float32)
    except Exception:
        pass
    return _forward_np(x, p).astype(np.float32)


# revision 3
# speedup vs baseline: 59.0187x; 59.0187x over previous
"""Data-parallel forward for nn_CerebroOriginal on 8 NeuronCores.

Sharding: pure data parallel per the problem's sharding hint — batch B=256 is
split 32/core across 8 cores, parameters are replicated. The reference's
per-tensor fake-quant (qdq) scales are GLOBAL maxima over the full batch, so a
cross-core max all-reduce (lax.pmax) runs at each quantization site; this
reproduces the reference scales exactly (max is order-independent and exact).
Everything else is per-sample and communication-free.

Falls back to single-process CPU execution if the device path fails, so the
kernel always returns a correct full-shape output.
"""
import numpy as np
import jax
import jax.numpy as jnp
from functools import partial

# Neuron's compiler auto-downcasts fp32 matmuls to bf16; the fake-quant
# rounding in this model amplifies that noise ~10x (rounding flips), so pin
# matmuls to full fp32 precision.
jax.config.update("jax_default_matmul_precision", "highest")

# CerebroOriginal dims (hardcoded per contract)
B, C, T, E, H, NB, NC = 256, 14, 8, 180, 5, 6, 4
DH = E // H
S = C * T
MLP = 4 * E
NCORES = 8
BL = B // NCORES  # 32 samples per core


def _qdq(x, axis_name):
    m = jnp.max(jnp.abs(x))
    if axis_name is not None:
        m = jax.lax.pmax(m, axis_name)
    s = jnp.maximum(m / 127.0, 1e-8)
    y = jnp.clip(jnp.round(x / s), -128.0, 127.0) * s
    return y


def _ln(x, g, b):
    m = jnp.mean(x, axis=-1, keepdims=True)
    v = jnp.var(x, axis=-1, keepdims=True)
    return (x - m) * jax.lax.rsqrt(v + 1e-5) * g + b


def _forward(x, p, axis_name=None):
    """Forward for a local batch shard. x: [b, C, T]."""
    b = x.shape[0]
    qdq = partial(_qdq, axis_name=axis_name)

    def qlinear(t, w, bias):
        return qdq(t) @ qdq(w).T + bias

    h = jnp.transpose(qdq(x), (0, 2, 1))
    h = qlinear(h, p["proj_w"], p["proj_b"])
    h = jnp.broadcast_to(h[:, None], (b, C, T, E)).reshape(b, S, E)

    for i in range(NB):
        r1 = h
        a = qdq(_ln(h, p["ln1_g"][i], p["ln1_b"][i])).reshape(b, C, T, E)
        if i % 2 == 0:
            a = a.reshape(b * C, T, E)
            L = T
        else:
            a = jnp.transpose(a, (0, 2, 1, 3)).reshape(b * T, C, E)
            L = C
        q = qlinear(a, p["wq"][i], p["bq"][i]).reshape(-1, L, H, DH)
        k = qlinear(a, p["wk"][i], p["bk"][i]).reshape(-1, L, H, DH)
        v = qlinear(a, p["wv"][i], p["bv"][i]).reshape(-1, L, H, DH)
        scores = jnp.einsum("nlhd,nmhd->nhlm", q, k) / jnp.sqrt(jnp.float32(DH))
        att = jax.nn.softmax(scores, axis=-1)
        o = jnp.einsum("nhlm,nmhd->nlhd", att, v).reshape(-1, L, E)
        o = qlinear(o, p["wo"][i], p["bo"][i])
        if i % 2 == 0:
            o = o.reshape(b, C, T, E)
        else:
            o = jnp.transpose(o.reshape(b, T, C, E), (0, 2, 1, 3))
        h = r1 + o.reshape(b, S, E)

        r2 = h
        m = qdq(_ln(h, p["ln2_g"][i], p["ln2_b"][i]))
        m = qlinear(m, p["fc1_w"][i], p["fc1_b"][i])
        m = qdq(jax.nn.gelu(m))
        m = qlinear(m, p["fc2_w"][i], p["fc2_b"][i])
        h = r2 + m

    h = qdq(_ln(h, p["fn_g"], p["fn_b"]))
    pooled = jnp.mean(h, axis=1)
    return qlinear(pooled, p["cls_w"], p["cls_b"])


_DEVICE_FN = None


def _get_device_fn():
    global _DEVICE_FN
    if _DEVICE_FN is not None:
        return _DEVICE_FN
    from jax.sharding import Mesh, PartitionSpec as P
    from jax.experimental.shard_map import shard_map

    devs = jax.devices()[:NCORES]
    mesh = Mesh(np.asarray(devs), ("i",))

    def shard_fwd(x, p):
        return _forward(x, p, axis_name="i")

    fn = jax.jit(
        shard_map(
            shard_fwd,
            mesh=mesh,
            in_specs=(P("i"), P()),
            out_specs=P("i"),
            check_rep=False,
        )
    )
    _DEVICE_FN = fn
    return fn


def _qdq_np(x):
    s = max(np.abs(x).max() / 127.0, 1e-8)
    return np.clip(np.round(x / s), -128.0, 127.0).astype(np.float32) * np.float32(s)


def _ln_np(x, g, b):
    m = x.mean(-1, keepdims=True)
    v = x.var(-1, keepdims=True)
    return (x - m) / np.sqrt(v + 1e-5) * g + b


def _gelu_np(x):
    # jax.nn.gelu default (approximate=True): tanh approximation
    return (0.5 * x * (1.0 + np.tanh(0.7978845608028654 * (x + 0.044715 * x**3)))).astype(np.float32)


def _forward_np(x, p):
    def qlinear(t, w, bias):
        return _qdq_np(t) @ _qdq_np(w).T + bias

    h = np.transpose(_qdq_np(x), (0, 2, 1))
    h = qlinear(h, p["proj_w"], p["proj_b"])
    h = np.broadcast_to(h[:, None], (B, C, T, E)).reshape(B, S, E).astype(np.float32)
    for i in range(NB):
        r1 = h
        a = _qdq_np(_ln_np(h, p["ln1_g"][i], p["ln1_b"][i])).reshape(B, C, T, E)
        if i % 2 == 0:
            a = a.reshape(B * C, T, E)
            L = T
        else:
            a = np.transpose(a, (0, 2, 1, 3)).reshape(B * T, C, E)
            L = C
        q = qlinear(a, p["wq"][i], p["bq"][i]).reshape(-1, L, H, DH)
        k = qlinear(a, p["wk"][i], p["bk"][i]).reshape(-1, L, H, DH)
        v = qlinear(a, p["wv"][i], p["bv"][i]).reshape(-1, L, H, DH)
        scores = np.einsum("nlhd,nmhd->nhlm", q, k) / np.sqrt(np.float32(DH))
        scores -= scores.max(-1, keepdims=True)
        e = np.exp(scores)
        att = e / e.sum(-1, keepdims=True)
        o = np.einsum("nhlm,nmhd->nlhd", att.astype(np.float32), v).reshape(-1, L, E)
        o = qlinear(o, p["wo"][i], p["bo"][i])
        if i % 2 == 0:
            o = o.reshape(B, C, T, E)
        else:
            o = np.transpose(o.reshape(B, T, C, E), (0, 2, 1, 3))
        h = r1 + o.reshape(B, S, E)
        r2 = h
        m = _qdq_np(_ln_np(h, p["ln2_g"][i], p["ln2_b"][i]))
        m = qlinear(m, p["fc1_w"][i], p["fc1_b"][i])
        m = _qdq_np(_gelu_np(m))
        m = qlinear(m, p["fc2_w"][i], p["fc2_b"][i])
        h = (r2 + m).astype(np.float32)
    h = _qdq_np(_ln_np(h, p["fn_g"], p["fn_b"]))
    pooled = h.mean(1)
    return qlinear(pooled, p["cls_w"], p["cls_b"])


def kernel(**inputs):
    x = np.asarray(inputs["x"], dtype=np.float32)
    p = {k: np.asarray(v) for k, v in inputs.items() if k != "x"}
    try:
        devs = jax.devices()
        if len(devs) >= NCORES and devs[0].platform != "cpu":
            fn = _get_device_fn()
            out = fn(jnp.asarray(x), jax.tree.map(jnp.asarray, p))
            out = np.asarray(jax.block_until_ready(out))
            if out.shape == (B, NC) and np.all(np.isfinite(out)):
                return out.astype(np.float32)
    except Exception:
        pass
    return _forward_np(x, p).astype(np.float32)
